# revision 35
# baseline (speedup 1.0000x reference)
"""Trainium2 Bass kernel for quaternion-algebra multi-head attention.

Math: algebra_linear(x, W, b) == x_flat @ M + b_flat where M[(n,j),(o,k)] =
sum_i C[i,j,k] W[o,n,i].  So the whole module is standard MHA with dense
1024x1024 projection matrices expanded on the host from the small algebra
weights.  Sharding: 8 cores = 2 batches x 4 head-groups (4 heads each).

Per-core device work (core c = (b, hg)), all matmuls bf16 with f32 psum:
  qT/kT [256, 2048] = Mq_shard^T-contraction against x^T (d on partitions)
  v_aug [2048, 4, 128] natural [s, d] layout per head: col 0 = ones
        (softmax denominator rides the context matmul), cols 64:128 = v
  S^T[sk, sq] = kT.T @ qT per head; head pairs run as two concurrent K=64
        matmuls in the top/bottom halves of the PE array (tile_position)
  expS = exp(S^T) on ScalarE, FD=1024 per instruction (no max subtraction:
        scores are bounded ~|4| and the mask is all-ones)
  ctx^T[128, sq] = v_aug.T @ expS -> row 0 denom, rows 64:128 context
  ctx_norm = ctx * recip(denom) (reciprocal_approx_fast + gpsimd
        partition_broadcast + DVE multiply)
  out^T[1024, 2048] partial = Mo_shard^T-contraction against ctx_norm
Host gathers: out[b] = sum_hg out_hg^T.T (+ bo).
The emission order software-pipelines the whole thing: scores/exp of block
k (a (sq-chunk, head-pair) pair) interleave with the ctx-accumulate of
block k-1 and with v/qk-projection/out-projection chains as PE filler, so
the ScalarE exp stream (the pacing engine) starts ~25us in and never
starves.
"""

import numpy as np
import ml_dtypes

B, S, E = 2, 2048, 1024
NB = 256          # algebra blocks
MD = 4            # quaternion dim
H = 16            # total heads
HD = 64           # head dim
H_PER = 4         # heads per core
D = 256           # head dims per core (H_PER * HD)
P = 128
NE = E // P       # 8 e-chunks
ND = D // P       # 2 d-chunks per core
SQ_T = 512
NSQ = S // SQ_T   # 4
SK_T = 128
NSK = S // SK_T   # 16
NDO = E // P      # 8 out-dim chunks
SCALE = 1.0 / np.sqrt(HD)

_QUAT_TABLE = [
    (0, 0, 0, 1.0), (0, 1, 1, 1.0), (0, 2, 2, 1.0), (0, 3, 3, 1.0),
    (1, 0, 1, 1.0), (2, 0, 2, 1.0), (3, 0, 3, 1.0),
    (1, 1, 0, -1.0), (2, 2, 0, -1.0), (3, 3, 0, -1.0),
    (1, 2, 3, 1.0), (2, 1, 3, -1.0),
    (2, 3, 1, 1.0), (3, 2, 1, -1.0),
    (3, 1, 2, 1.0), (1, 3, 2, -1.0),
]


def _quat_C():
    C = np.zeros((4, 4, 4), dtype=np.float32)
    for i, j, k, s in _QUAT_TABLE:
        C[i, j, k] = s
    return C


def _expand(W, C):
    # W [NB, NB, 4] -> dense [E, E]:  y_flat = x_flat @ M
    Wm = np.einsum('oni,ijk->onjk', W.astype(np.float32), C)
    return np.ascontiguousarray(Wm.transpose(1, 2, 0, 3).reshape(E, E))


def _build_graph(with_qk_bias, with_v_bias, with_mask):
    import concourse.bacc as bacc
    import concourse.tile as tile
    import concourse.mybir as mybir

    f32 = mybir.dt.float32
    bf16 = mybir.dt.bfloat16
    Exp = mybir.ActivationFunctionType.Exp
    Identity = mybir.ActivationFunctionType.Identity

    nc = bacc.Bacc("TRN2", target_bir_lowering=False, debug=False, num_devices=8)

    xt_d = nc.dram_tensor("xt", [E, S], bf16, kind="ExternalInput").ap()
    wq_d = nc.dram_tensor("wq", [E, D], bf16, kind="ExternalInput").ap()
    wk_d = nc.dram_tensor("wk", [E, D], bf16, kind="ExternalInput").ap()
    wv_d = nc.dram_tensor("wv", [E, D], bf16, kind="ExternalInput").ap()
    wo_d = nc.dram_tensor("wo", [D, E], bf16, kind="ExternalInput").ap()
    out_d = nc.dram_tensor("out", [E, S], f32, kind="ExternalOutput").ap()
    if with_qk_bias:
        bq_d = nc.dram_tensor("bq", [D], f32, kind="ExternalInput").ap()
        bk_d = nc.dram_tensor("bk", [D], f32, kind="ExternalInput").ap()
    if with_v_bias:
        bv_d = nc.dram_tensor("bv", [D], bf16, kind="ExternalInput").ap()
    if with_mask:
        maskT_d = nc.dram_tensor("maskT", [S, S], bf16, kind="ExternalInput").ap()

    with tile.TileContext(nc) as tc:
        import contextlib
        with nc.allow_low_precision(reason="float32r rounding of matmul operands"), \
                contextlib.ExitStack() as ctx:
            sing = ctx.enter_context(tc.tile_pool(name="sing", bufs=1))
            psum = ctx.enter_context(tc.tile_pool(name="psum", bufs=1, space="PSUM"))
            work = ctx.enter_context(tc.tile_pool(name="work", bufs=1))

            # ---- persistent SBUF tiles ----
            xt_sb = [
                sing.tile([P, S], bf16, name=f"xt{e}", tag=f"xt{e}")
                for e in range(NE)
            ]
            wq_sb = sing.tile([P, NE, D], bf16, name="wq_sb", tag="wq_sb")
            wk_sb = sing.tile([P, NE, D], bf16, name="wk_sb", tag="wk_sb")
            wv_sb = sing.tile([P, NE, D], bf16, name="wv_sb", tag="wv_sb")
            wo_sb = sing.tile([P, ND, E], bf16, name="wo_sb", tag="wo_sb")
            qT_sb = sing.tile([P, ND, S], bf16, name="qT_sb", tag="qT_sb")
            kT_sb = sing.tile([P, ND, S], bf16, name="kT_sb", tag="kT_sb")
            v_aug = sing.tile([P, NSK, H_PER, P], bf16, name="v_aug", tag="v_aug")

            # ---- input DMAs ----
            nc.sync.dma_start(wk_sb, wk_d.rearrange("(ko p) d -> p ko d", p=P))
            nc.sync.dma_start(wq_sb, wq_d.rearrange("(ko p) d -> p ko d", p=P))
            for e in range(NE):
                nc.sync.dma_start(xt_sb[e], xt_d[e * P:(e + 1) * P, :])
            nc.sync.dma_start(wv_sb, wv_d.rearrange("(ko p) d -> p ko d", p=P))
            nc.sync.dma_start(wo_sb, wo_d.rearrange("(dk p) o -> p dk o", p=P))

            import ml_dtypes
            ones_init = nc.inline_tensor(
                np.ones((P, NSK, H_PER, P), ml_dtypes.bfloat16), name="ones_init").ap()
            ones_row = nc.inline_tensor(
                np.ones((1, P), ml_dtypes.bfloat16), name="ones_row").ap()
            nc.sync.dma_start(v_aug, ones_init)

            if with_qk_bias:
                bq_sb = sing.tile([P, ND], f32, name="bq_sb", tag="bq_sb")
                bk_sb = sing.tile([P, ND], f32, name="bk_sb", tag="bk_sb")
                nc.sync.dma_start(bq_sb, bq_d.rearrange("(dk p) -> p dk", p=P))
                nc.sync.dma_start(bk_sb, bk_d.rearrange("(dk p) -> p dk", p=P))
            if with_v_bias:
                bv_row = sing.tile([1, D], bf16, name="bv_row", tag="bv_row")
                ones_r = sing.tile([1, P], bf16, name="ones_r", tag="ones_r")
                nc.sync.dma_start(bv_row, bv_d[None, :])
                nc.sync.dma_start(ones_r, ones_row)

            # ---- helper emitters ----
            def v_chain(st):
                pv = psum.tile([P, D], f32, name="pv", tag="mm", bufs=2)
                n_acc = NE + (1 if with_v_bias else 0)
                for e in range(NE):
                    nc.tensor.matmul(
                        pv,
                        lhsT=xt_sb[e][:, st * P:(st + 1) * P],
                        rhs=wv_sb[:, e, :],
                        start=(e == 0),
                        stop=(e == n_acc - 1),
                    )
                if with_v_bias:
                    nc.tensor.matmul(pv, lhsT=ones_r, rhs=bv_row,
                                     start=False, stop=True)
                for h in range(H_PER):
                    nc.vector.tensor_copy(
                        v_aug[:, st, h, HD:2 * HD], pv[:, h * HD:(h + 1) * HD])

            def scores_exp_step(si, dk, sk, ex_tag, ex_bufs):
                sq = slice(si * SQ_T, (si + 1) * SQ_T)
                ps = psum.tile([P, 2, SQ_T], f32, name="ps", tag="sc", bufs=2)
                for j in range(2):
                    po = j * HD
                    nc.tensor.matmul(
                        ps[:, j, :],
                        lhsT=kT_sb[po:po + HD, dk, sk * SK_T:(sk + 1) * SK_T],
                        rhs=qT_sb[po:po + HD, dk, sq],
                        start=True, stop=True,
                        tile_position=(po, 0),
                    )
                ex = work.tile([P, 2, SQ_T], bf16, name="ex", tag=ex_tag, bufs=ex_bufs)
                nc.scalar.activation(ex, ps, Exp)
                if with_mask:
                    mt = work.tile([P, SQ_T], bf16, name="mt", tag="mt", bufs=4)
                    nc.sync.dma_start(
                        mt, maskT_d[sk * SK_T:(sk + 1) * SK_T, sq])
                    nc.vector.tensor_mul(
                        ex, ex, mt[:, None, :].to_broadcast([P, 2, SQ_T]))
                return ex

            def ctx_step(dk, sk, pcs, ex):
                for j in range(2):
                    nc.tensor.matmul(
                        pcs[j], lhsT=v_aug[:, sk, 2 * dk + j, :],
                        rhs=ex[:, j, :],
                        start=(sk == 0), stop=(sk == NSK - 1),
                    )

            def alloc_pcs():
                return [
                    psum.tile([P, SQ_T], f32, name=f"pc{j}", tag="pc", bufs=2)
                    for j in range(2)
                ]

            def normalize(dk, pcs, ctxT, tail=False):
                # copy ctx rows + reciprocal first so the psum pair frees
                # quickly (the next block's ctx chain waits on these slots);
                # in the tail run complete per-head chains to shorten the
                # critical path into the final out-projection
                cus, recfs = [], []
                order = ([(0, "cu"), (0, "rf"), (0, "bm"), (1, "cu"),
                          (1, "rf"), (1, "bm")] if tail else
                         [(0, "cu"), (0, "rf"), (1, "cu"), (1, "rf"),
                          (0, "bm"), (1, "bm")])
                for j, step in order:
                    pc, po = pcs[j], j * HD
                    if step == "cu":
                        cu = work.tile([HD, SQ_T], f32, name="cu", tag="cu", bufs=3)
                        nc.vector.tensor_copy(cu, pc[HD:2 * HD, :])
                        cus.append(cu)
                    elif step == "rf":
                        recf = work.tile([1, SQ_T], f32, name="recf", tag="recf", bufs=3)
                        nc.vector.reciprocal_approx_fast(recf, pc[0:1, :])
                        recfs.append(recf)
                    else:
                        bca = work.tile([HD, SQ_T], f32, name="bca", tag="bca", bufs=3)
                        nc.gpsimd.partition_broadcast(bca, recfs[j])
                        nc.vector.tensor_mul(ctxT[po:po + HD, dk, :], cus[j], bca)

            # ---- emission schedule: staggered-start 2-stage pipeline ----
            from collections import deque
            ctxTs = {
                si: work.tile(
                    [P, ND, SQ_T], bf16, name=f"ctxT{si}", tag="ctxT", bufs=NSQ)
                for si in range(NSQ)
            }

            def qk_chain(dk, wsb, dst, bias_sb, si):
                pp = psum.tile([P, SQ_T], f32, name="pp", tag="mm", bufs=2)
                for e in range(NE):
                    nc.tensor.matmul(
                        pp,
                        lhsT=wsb[:, e, dk * P:(dk + 1) * P],
                        rhs=xt_sb[e][:, si * SQ_T:(si + 1) * SQ_T],
                        start=(e == 0),
                        stop=(e == NE - 1),
                    )
                dslice = dst[:, dk, si * SQ_T:(si + 1) * SQ_T]
                if with_qk_bias:
                    bb = bq_sb if bias_sb == "bq_sb" else bk_sb
                    nc.scalar.activation(
                        dslice, pp, Identity, bias=bb[:, dk:dk + 1])
                else:
                    nc.vector.tensor_copy(dslice, pp)

            def outproj_unit(si, do, ctxT, tail=False):
                sq = slice(si * SQ_T, (si + 1) * SQ_T)
                # tail units alternate psum tags for a deeper pipeline
                # (score slots are free once the exp stream has ended)
                ptag = ("sc" if do % 2 else "mm") if tail else "mm"
                pu = psum.tile([P, SQ_T], f32, name="pu", tag=ptag, bufs=2)
                for dkk in range(ND):
                    nc.tensor.matmul(
                        pu,
                        lhsT=wo_sb[:, dkk, do * P:(do + 1) * P],
                        rhs=ctxT[:, dkk, :],
                        start=(dkk == 0), stop=(dkk == ND - 1),
                    )
                ot = work.tile([P, SQ_T], f32, name="ot", tag="ot", bufs=3)
                if tail:
                    nc.scalar.copy(ot, pu)
                else:
                    nc.vector.tensor_copy(ot, pu)
                nc.sync.dma_start(out_d[do * P:(do + 1) * P, sq], ot)

            filler = deque()

            def pop_filler():
                if filler:
                    filler.popleft()()

            def flush_filler():
                while filler:
                    filler.popleft()()

            def mk(fn, *a):
                return lambda: fn(*a)

            blocks = [(0, 0), (1, 0), (2, 0), (0, 1),
                      (1, 1), (2, 1), (3, 0), (3, 1)]
            ex_store = {}
            pcs_store = {}

            # block-0 fillers: first half of v chains + remaining q-dk0 chains
            b0f = [mk(v_chain, st) for st in range(8)]
            for i, s in enumerate(range(1, NSQ)):
                b0f.insert(2 * i + 1, mk(qk_chain, 0, wq_sb, qT_sb, "bq_sb", s))
            filler.extend(b0f)

            for bi, (si, dk) in enumerate(blocks):
                if bi == 1:
                    b1f = [mk(v_chain, st) for st in range(8, NSK)]
                    ins_ = [mk(qk_chain, 1, wk_sb, kT_sb, "bk_sb", s) for s in range(2)]
                    for i, it in enumerate(ins_):
                        b1f.insert(2 * i + 1, it)
                    filler.extend(b1f)
                if bi == 2:
                    filler.append(mk(qk_chain, 1, wk_sb, kT_sb, "bk_sb", 2))
                    filler.append(mk(qk_chain, 1, wk_sb, kT_sb, "bk_sb", 3))
                    filler.append(mk(qk_chain, 1, wq_sb, qT_sb, "bq_sb", 0))
                if bi == 3:
                    for s in range(1, NSQ):
                        filler.append(mk(qk_chain, 1, wq_sb, qT_sb, "bq_sb", s))
                prev = blocks[bi - 1] if bi > 0 else None
                ex_store[(si, dk)] = []
                for sk in range(NSK):
                    if bi == 0:
                        if sk == 0:
                            # interleave the two chains e-wise: both finish
                            # right after the last xt chunk arrives
                            ppk = psum.tile([P, SQ_T], f32, name="ppk", tag="mm", bufs=2)
                            ppq = psum.tile([P, SQ_T], f32, name="ppq", tag="mm", bufs=2)
                            for e in range(NE):
                                nc.tensor.matmul(
                                    ppk, lhsT=wk_sb[:, e, 0:P],
                                    rhs=xt_sb[e][:, 0:SQ_T],
                                    start=(e == 0), stop=(e == NE - 1))
                                nc.tensor.matmul(
                                    ppq, lhsT=wq_sb[:, e, 0:P],
                                    rhs=xt_sb[e][:, 0:SQ_T],
                                    start=(e == 0), stop=(e == NE - 1))
                            nc.vector.tensor_copy(kT_sb[:, 0, 0:SQ_T], ppk)
                            nc.vector.tensor_copy(qT_sb[:, 0, 0:SQ_T], ppq)
                        elif sk % NSQ == 0:
                            qk_chain(0, wk_sb, kT_sb, "bk_sb", sk // NSQ)
                    ex_store[(si, dk)].append(
                        scores_exp_step(si, dk, sk, "exs", 2 * NSK + 2))
                    if prev is not None:
                        if sk == 0:
                            pcs_store[prev] = alloc_pcs()
                        ctx_step(prev[1], sk, pcs_store[prev], ex_store[prev][sk])
                    pop_filler()
                if bi <= 1:
                    flush_filler()
                if prev is not None:
                    normalize(prev[1], pcs_store[prev], ctxTs[prev[0]])
                    del ex_store[prev]
                    if prev[1] == 1:
                        for do in range(NDO):
                            filler.append(mk(outproj_unit, prev[0], do, ctxTs[prev[0]]))
            # tail: last block's ctx on freed score slots, outproj(2)
            # interleaved, then normalize + outproj(3)
            last = blocks[-1]
            pcs_store[last] = [
                psum.tile([P, SQ_T], f32, name=f"pcl{j}", tag="sc", bufs=2)
                for j in range(2)
            ]
            for sk in range(NSK):
                ctx_step(last[1], sk, pcs_store[last], ex_store[last][sk])
                pop_filler()
            flush_filler()
            normalize(last[1], pcs_store[last], ctxTs[last[0]], tail=True)
            for do in range(NDO):
                outproj_unit(last[0], do, ctxTs[last[0]], tail=True)

    nc.compile()
    return nc


_GRAPH_CACHE = {}


def kernel(x, mask, Wq, bq, Wk, bk, Wv, bv, Wo, bo):
    from concourse.bass_utils import run_bass_kernel_spmd

    x = np.asarray(x, dtype=np.float32)
    mask = np.asarray(mask)
    C = _quat_C()
    Mq = _expand(np.asarray(Wq), C) * SCALE
    Mk = _expand(np.asarray(Wk), C)
    Mv = _expand(np.asarray(Wv), C)
    Mo = _expand(np.asarray(Wo), C)
    bq_f = np.asarray(bq, np.float32).reshape(-1) * SCALE
    bk_f = np.asarray(bk, np.float32).reshape(-1)
    bv_f = np.asarray(bv, np.float32).reshape(-1)
    bo_f = np.asarray(bo, np.float32).reshape(-1)

    with_qk_bias = bool(np.any(bq_f) or np.any(bk_f))
    with_v_bias = bool(np.any(bv_f))
    with_mask = bool(np.any(np.asarray(mask) == 0))

    key = (with_qk_bias, with_v_bias, with_mask)
    if key not in _GRAPH_CACHE:
        _GRAPH_CACHE[key] = _build_graph(*key)
    nc = _GRAPH_CACHE[key]

    if with_mask:
        maskT = np.ascontiguousarray(
            np.broadcast_to(mask, (1, 1, S, S))[0, 0].T.astype(ml_dtypes.bfloat16))

    in_maps = []
    for core in range(8):
        b, hg = core // 4, core % 4
        cs = slice(hg * D, (hg + 1) * D)
        m = {
            "xt": np.ascontiguousarray(x[b].T.astype(ml_dtypes.bfloat16)),
            "wq": np.ascontiguousarray(Mq[:, cs].astype(ml_dtypes.bfloat16)),
            "wk": np.ascontiguousarray(Mk[:, cs].astype(ml_dtypes.bfloat16)),
            "wv": np.ascontiguousarray(Mv[:, cs].astype(ml_dtypes.bfloat16)),
            "wo": np.ascontiguousarray(Mo[cs, :].astype(ml_dtypes.bfloat16)),
        }
        if with_qk_bias:
            m["bq"] = np.ascontiguousarray(bq_f[cs])
            m["bk"] = np.ascontiguousarray(bk_f[cs])
        if with_v_bias:
            m["bv"] = np.ascontiguousarray(bv_f[cs].astype(ml_dtypes.bfloat16))
        if with_mask:
            m["maskT"] = maskT
        in_maps.append(m)

    res = run_bass_kernel_spmd(nc, in_maps, core_ids=list(range(8))).results

    out = np.zeros((B, S, E), dtype=np.float32)
    for core in range(8):
        b = core // 4
        out[b] += res[core]["out"].T
    out += bo_f
    return out


# revision 37
# speedup vs baseline: 1.1696x; 1.1696x over previous
"""Trainium2 Bass kernel for quaternion-algebra multi-head attention.

Math: algebra_linear(x, W, b) == x_flat @ M + b_flat where M[(n,j),(o,k)] =
sum_i C[i,j,k] W[o,n,i].  So the whole module is standard MHA with dense
1024x1024 projection matrices expanded on the host from the small algebra
weights.  Sharding: 8 cores = 2 batches x 4 head-groups (4 heads each).

Per-core device work (core c = (b, hg)), all matmuls bf16 with f32 psum:
  qT/kT [256, 2048] = Mq_shard^T-contraction against x^T (d on partitions)
  v_aug [2048, 4, 128] natural [s, d] layout per head: col 0 = ones
        (softmax denominator rides the context matmul), cols 64:128 = v
  S^T[sk, sq] = kT.T @ qT per head; head pairs run as two concurrent K=64
        matmuls in the top/bottom halves of the PE array (tile_position)
  expS = exp(S^T) on ScalarE, FD=1024 per instruction (no max subtraction:
        scores are bounded ~|4| and the mask is all-ones)
  ctx^T[128, sq] = v_aug.T @ expS -> row 0 denom, rows 64:128 context
  ctx_norm = ctx * recip(denom) (reciprocal_approx_fast + gpsimd
        partition_broadcast + DVE multiply)
  out^T[1024, 2048] partial = Mo_shard^T-contraction against ctx_norm
Host gathers: out[b] = sum_hg out_hg^T.T (+ bo).
The emission order software-pipelines the whole thing: scores/exp of block
k (a (sq-chunk, head-pair) pair) interleave with the ctx-accumulate of
block k-1 and with v/qk-projection/out-projection chains as PE filler, so
the ScalarE exp stream (the pacing engine) starts ~25us in and never
starves.
"""

import numpy as np
import ml_dtypes

B, S, E = 2, 2048, 1024
NB = 256          # algebra blocks
MD = 4            # quaternion dim
H = 16            # total heads
HD = 64           # head dim
H_PER = 4         # heads per core
D = 256           # head dims per core (H_PER * HD)
P = 128
NE = E // P       # 8 e-chunks
ND = D // P       # 2 d-chunks per core
SQ_T = 512
NSQ = S // SQ_T   # 4
SK_T = 128
NSK = S // SK_T   # 16
NDO = E // P      # 8 out-dim chunks
SCALE = 1.0 / np.sqrt(HD)

_QUAT_TABLE = [
    (0, 0, 0, 1.0), (0, 1, 1, 1.0), (0, 2, 2, 1.0), (0, 3, 3, 1.0),
    (1, 0, 1, 1.0), (2, 0, 2, 1.0), (3, 0, 3, 1.0),
    (1, 1, 0, -1.0), (2, 2, 0, -1.0), (3, 3, 0, -1.0),
    (1, 2, 3, 1.0), (2, 1, 3, -1.0),
    (2, 3, 1, 1.0), (3, 2, 1, -1.0),
    (3, 1, 2, 1.0), (1, 3, 2, -1.0),
]


def _quat_C():
    C = np.zeros((4, 4, 4), dtype=np.float32)
    for i, j, k, s in _QUAT_TABLE:
        C[i, j, k] = s
    return C


def _expand(W, C):
    # W [NB, NB, 4] -> dense [E, E]:  y_flat = x_flat @ M
    Wm = np.einsum('oni,ijk->onjk', W.astype(np.float32), C)
    return np.ascontiguousarray(Wm.transpose(1, 2, 0, 3).reshape(E, E))


def _build_graph(with_qk_bias, with_v_bias, with_mask):
    import concourse.bacc as bacc
    import concourse.tile as tile
    import concourse.mybir as mybir

    f32 = mybir.dt.float32
    bf16 = mybir.dt.bfloat16
    Exp = mybir.ActivationFunctionType.Exp
    Identity = mybir.ActivationFunctionType.Identity

    nc = bacc.Bacc("TRN2", target_bir_lowering=False, debug=False, num_devices=8)

    xt_d = nc.dram_tensor("xt", [E, S], bf16, kind="ExternalInput").ap()
    wq_d = nc.dram_tensor("wq", [E, D], bf16, kind="ExternalInput").ap()
    wk_d = nc.dram_tensor("wk", [E, D], bf16, kind="ExternalInput").ap()
    wv_d = nc.dram_tensor("wv", [E, D], bf16, kind="ExternalInput").ap()
    wo_d = nc.dram_tensor("wo", [D, E], bf16, kind="ExternalInput").ap()
    out_d = nc.dram_tensor("out", [E, S], f32, kind="ExternalOutput").ap()
    if with_qk_bias:
        bq_d = nc.dram_tensor("bq", [D], f32, kind="ExternalInput").ap()
        bk_d = nc.dram_tensor("bk", [D], f32, kind="ExternalInput").ap()
    if with_v_bias:
        bv_d = nc.dram_tensor("bv", [D], bf16, kind="ExternalInput").ap()
    if with_mask:
        maskT_d = nc.dram_tensor("maskT", [S, S], bf16, kind="ExternalInput").ap()

    with tile.TileContext(nc) as tc:
        import contextlib
        with nc.allow_low_precision(reason="float32r rounding of matmul operands"), \
                contextlib.ExitStack() as ctx:
            sing = ctx.enter_context(tc.tile_pool(name="sing", bufs=1))
            psum = ctx.enter_context(tc.tile_pool(name="psum", bufs=1, space="PSUM"))
            work = ctx.enter_context(tc.tile_pool(name="work", bufs=1))

            # ---- persistent SBUF tiles ----
            xt_sb = [
                sing.tile([P, S], bf16, name=f"xt{e}", tag=f"xt{e}")
                for e in range(NE)
            ]
            wq_sb = sing.tile([P, NE, D], bf16, name="wq_sb", tag="wq_sb")
            wk_sb = sing.tile([P, NE, D], bf16, name="wk_sb", tag="wk_sb")
            wv_sb = sing.tile([P, NE, D], bf16, name="wv_sb", tag="wv_sb")
            wo_sb = sing.tile([P, ND, E], bf16, name="wo_sb", tag="wo_sb")
            qT_sb = sing.tile([P, ND, S], bf16, name="qT_sb", tag="qT_sb")
            kT_sb = sing.tile([P, ND, S], bf16, name="kT_sb", tag="kT_sb")
            v_aug = sing.tile([P, NSK, H_PER, P], bf16, name="v_aug", tag="v_aug")

            # ---- input DMAs ----
            nc.sync.dma_start(wk_sb, wk_d.rearrange("(ko p) d -> p ko d", p=P))
            nc.sync.dma_start(wq_sb, wq_d.rearrange("(ko p) d -> p ko d", p=P))
            for e in range(NE):
                nc.sync.dma_start(xt_sb[e], xt_d[e * P:(e + 1) * P, :])
            nc.sync.dma_start(wv_sb, wv_d.rearrange("(ko p) d -> p ko d", p=P))
            nc.sync.dma_start(wo_sb, wo_d.rearrange("(dk p) o -> p dk o", p=P))

            import ml_dtypes
            ones_init = nc.inline_tensor(
                np.ones((P, NSK, H_PER, P), ml_dtypes.bfloat16), name="ones_init").ap()
            ones_row = nc.inline_tensor(
                np.ones((1, P), ml_dtypes.bfloat16), name="ones_row").ap()
            nc.sync.dma_start(v_aug, ones_init)

            if with_qk_bias:
                bq_sb = sing.tile([P, ND], f32, name="bq_sb", tag="bq_sb")
                bk_sb = sing.tile([P, ND], f32, name="bk_sb", tag="bk_sb")
                nc.sync.dma_start(bq_sb, bq_d.rearrange("(dk p) -> p dk", p=P))
                nc.sync.dma_start(bk_sb, bk_d.rearrange("(dk p) -> p dk", p=P))
            if with_v_bias:
                bv_row = sing.tile([1, D], bf16, name="bv_row", tag="bv_row")
                ones_r = sing.tile([1, P], bf16, name="ones_r", tag="ones_r")
                nc.sync.dma_start(bv_row, bv_d[None, :])
                nc.sync.dma_start(ones_r, ones_row)

            # ---- helper emitters ----
            def v_chain(st):
                pv = psum.tile([P, D], f32, name="pv", tag="mm", bufs=2)
                n_acc = NE + (1 if with_v_bias else 0)
                for e in range(NE):
                    nc.tensor.matmul(
                        pv,
                        lhsT=xt_sb[e][:, st * P:(st + 1) * P],
                        rhs=wv_sb[:, e, :],
                        start=(e == 0),
                        stop=(e == n_acc - 1),
                    )
                if with_v_bias:
                    nc.tensor.matmul(pv, lhsT=ones_r, rhs=bv_row,
                                     start=False, stop=True)
                for h in range(H_PER):
                    nc.vector.tensor_copy(
                        v_aug[:, st, h, HD:2 * HD], pv[:, h * HD:(h + 1) * HD])

            def scores_exp_step(si, dk, sk, ex_tag, ex_bufs):
                sq = slice(si * SQ_T, (si + 1) * SQ_T)
                ps = psum.tile([P, 2, SQ_T], f32, name="ps", tag="sc", bufs=2)
                for j in range(2):
                    po = j * HD
                    nc.tensor.matmul(
                        ps[:, j, :],
                        lhsT=kT_sb[po:po + HD, dk, sk * SK_T:(sk + 1) * SK_T],
                        rhs=qT_sb[po:po + HD, dk, sq],
                        start=True, stop=True,
                        tile_position=(po, 0),
                    )
                ex = work.tile([P, 2, SQ_T], bf16, name="ex", tag=ex_tag, bufs=ex_bufs)
                nc.scalar.activation(ex, ps, Exp)
                if with_mask:
                    mt = work.tile([P, SQ_T], bf16, name="mt", tag="mt", bufs=4)
                    nc.sync.dma_start(
                        mt, maskT_d[sk * SK_T:(sk + 1) * SK_T, sq])
                    nc.vector.tensor_mul(
                        ex, ex, mt[:, None, :].to_broadcast([P, 2, SQ_T]))
                return ex

            def ctx_step(dk, sk, pcs, ex):
                for j in range(2):
                    nc.tensor.matmul(
                        pcs[j], lhsT=v_aug[:, sk, 2 * dk + j, :],
                        rhs=ex[:, j, :],
                        start=(sk == 0), stop=(sk == NSK - 1),
                    )

            def alloc_pcs():
                return [
                    psum.tile([P, SQ_T], f32, name=f"pc{j}", tag="pc", bufs=2)
                    for j in range(2)
                ]

            def normalize(dk, pcs, ctxT, tail=False):
                # copy ctx rows + reciprocal first so the psum pair frees
                # quickly (the next block's ctx chain waits on these slots);
                # in the tail run complete per-head chains to shorten the
                # critical path into the final out-projection
                cus, recfs = [], []
                order = ([(0, "cu"), (0, "rf"), (0, "bm"), (1, "cu"),
                          (1, "rf"), (1, "bm")] if tail else
                         [(0, "cu"), (0, "rf"), (1, "cu"), (1, "rf"),
                          (0, "bm"), (1, "bm")])
                for j, step in order:
                    pc, po = pcs[j], j * HD
                    if step == "cu":
                        cu = work.tile([HD, SQ_T], f32, name="cu", tag="cu", bufs=3)
                        nc.vector.tensor_copy(cu, pc[HD:2 * HD, :])
                        cus.append(cu)
                    elif step == "rf":
                        recf = work.tile([1, SQ_T], f32, name="recf", tag="recf", bufs=3)
                        nc.vector.reciprocal_approx_fast(recf, pc[0:1, :])
                        recfs.append(recf)
                    else:
                        bca = work.tile([HD, SQ_T], f32, name="bca", tag="bca", bufs=3)
                        nc.gpsimd.partition_broadcast(bca, recfs[j])
                        nc.vector.tensor_mul(ctxT[po:po + HD, dk, :], cus[j], bca)

            # ---- emission schedule: staggered-start 2-stage pipeline ----
            from collections import deque
            ctxTs = {
                si: work.tile(
                    [P, ND, SQ_T], bf16, name=f"ctxT{si}", tag="ctxT", bufs=NSQ)
                for si in range(NSQ)
            }

            def qk_chain(dk, wsb, dst, bias_sb, si):
                pp = psum.tile([P, SQ_T], f32, name="pp", tag="mm", bufs=2)
                for e in range(NE):
                    nc.tensor.matmul(
                        pp,
                        lhsT=wsb[:, e, dk * P:(dk + 1) * P],
                        rhs=xt_sb[e][:, si * SQ_T:(si + 1) * SQ_T],
                        start=(e == 0),
                        stop=(e == NE - 1),
                    )
                dslice = dst[:, dk, si * SQ_T:(si + 1) * SQ_T]
                if with_qk_bias:
                    bb = bq_sb if bias_sb == "bq_sb" else bk_sb
                    nc.scalar.activation(
                        dslice, pp, Identity, bias=bb[:, dk:dk + 1])
                else:
                    nc.vector.tensor_copy(dslice, pp)

            def outproj_unit(si, do, ctxT, tail=False):
                sq = slice(si * SQ_T, (si + 1) * SQ_T)
                # tail units alternate psum tags for a deeper pipeline
                # (score slots are free once the exp stream has ended)
                ptag = ("sc" if do % 2 else "mm") if tail else "mm"
                pu = psum.tile([P, SQ_T], f32, name="pu", tag=ptag, bufs=2)
                for dkk in range(ND):
                    nc.tensor.matmul(
                        pu,
                        lhsT=wo_sb[:, dkk, do * P:(do + 1) * P],
                        rhs=ctxT[:, dkk, :],
                        start=(dkk == 0), stop=(dkk == ND - 1),
                    )
                ot = work.tile([P, SQ_T], f32, name="ot", tag="ot", bufs=3)
                if tail:
                    nc.scalar.copy(ot, pu)
                else:
                    nc.vector.tensor_copy(ot, pu)
                nc.sync.dma_start(out_d[do * P:(do + 1) * P, sq], ot)

            filler = deque()

            def pop_filler():
                if filler:
                    filler.popleft()()

            def flush_filler():
                while filler:
                    filler.popleft()()

            def mk(fn, *a):
                return lambda: fn(*a)

            blocks = [(0, 0), (1, 0), (2, 0), (0, 1),
                      (1, 1), (2, 1), (3, 0), (3, 1)]
            ex_store = {}
            pcs_store = {}

            # block-0 fillers: first half of v chains + remaining q-dk0 chains
            b0f = [mk(v_chain, st) for st in range(8)]
            for i, s in enumerate(range(1, NSQ)):
                b0f.insert(2 * i + 1, mk(qk_chain, 0, wq_sb, qT_sb, "bq_sb", s))
            filler.extend(b0f)

            for bi, (si, dk) in enumerate(blocks):
                if bi == 1:
                    b1f = [mk(v_chain, st) for st in range(8, NSK)]
                    ins_ = [mk(qk_chain, 1, wk_sb, kT_sb, "bk_sb", s) for s in range(2)]
                    for i, it in enumerate(ins_):
                        b1f.insert(2 * i + 1, it)
                    filler.extend(b1f)
                if bi == 2:
                    filler.append(mk(qk_chain, 1, wk_sb, kT_sb, "bk_sb", 2))
                    filler.append(mk(qk_chain, 1, wk_sb, kT_sb, "bk_sb", 3))
                    filler.append(mk(qk_chain, 1, wq_sb, qT_sb, "bq_sb", 0))
                if bi == 3:
                    for s in range(1, NSQ):
                        filler.append(mk(qk_chain, 1, wq_sb, qT_sb, "bq_sb", s))
                prev = blocks[bi - 1] if bi > 0 else None
                ex_store[(si, dk)] = []
                for sk in range(NSK):
                    if bi == 0:
                        if sk == 0:
                            # interleave the two chains e-wise: both finish
                            # right after the last xt chunk arrives
                            ppk = psum.tile([P, SQ_T], f32, name="ppk", tag="mm", bufs=2)
                            ppq = psum.tile([P, SQ_T], f32, name="ppq", tag="mm", bufs=2)
                            for e in range(NE):
                                nc.tensor.matmul(
                                    ppk, lhsT=wk_sb[:, e, 0:P],
                                    rhs=xt_sb[e][:, 0:SQ_T],
                                    start=(e == 0), stop=(e == NE - 1))
                                nc.tensor.matmul(
                                    ppq, lhsT=wq_sb[:, e, 0:P],
                                    rhs=xt_sb[e][:, 0:SQ_T],
                                    start=(e == 0), stop=(e == NE - 1))
                            nc.vector.tensor_copy(kT_sb[:, 0, 0:SQ_T], ppk)
                            nc.vector.tensor_copy(qT_sb[:, 0, 0:SQ_T], ppq)
                        elif sk % NSQ == 0:
                            qk_chain(0, wk_sb, kT_sb, "bk_sb", sk // NSQ)
                    ex_store[(si, dk)].append(
                        scores_exp_step(si, dk, sk, "exs", 2 * NSK + 2))
                    if prev is not None:
                        if sk == 0:
                            pcs_store[prev] = alloc_pcs()
                        ctx_step(prev[1], sk, pcs_store[prev], ex_store[prev][sk])
                    pop_filler()
                if bi <= 1:
                    flush_filler()
                if prev is not None:
                    normalize(prev[1], pcs_store[prev], ctxTs[prev[0]])
                    del ex_store[prev]
                    if prev[1] == 1:
                        for do in range(NDO):
                            filler.append(mk(outproj_unit, prev[0], do, ctxTs[prev[0]]))
            # tail: last block's ctx on freed score slots, outproj(2)
            # interleaved, then normalize + outproj(3)
            last = blocks[-1]
            pcs_store[last] = [
                psum.tile([P, SQ_T], f32, name=f"pcl{j}", tag="sc", bufs=2)
                for j in range(2)
            ]
            for sk in range(NSK):
                ctx_step(last[1], sk, pcs_store[last], ex_store[last][sk])
                pop_filler()
            flush_filler()
            normalize(last[1], pcs_store[last], ctxTs[last[0]], tail=True)
            for do in range(NDO):
                outproj_unit(last[0], do, ctxTs[last[0]], tail=True)

    nc.compile()
    return nc


_GRAPH_CACHE = {}


def kernel(x, mask, Wq, bq, Wk, bk, Wv, bv, Wo, bo):
    from concourse.bass_utils import run_bass_kernel_spmd

    x = np.asarray(x, dtype=np.float32)
    mask = np.asarray(mask)
    C = _quat_C()
    Mq = _expand(np.asarray(Wq), C) * SCALE
    Mk = _expand(np.asarray(Wk), C)
    Mv = _expand(np.asarray(Wv), C)
    Mo = _expand(np.asarray(Wo), C)
    bq_f = np.asarray(bq, np.float32).reshape(-1) * SCALE
    bk_f = np.asarray(bk, np.float32).reshape(-1)
    bv_f = np.asarray(bv, np.float32).reshape(-1)
    bo_f = np.asarray(bo, np.float32).reshape(-1)

    with_qk_bias = bool(np.any(bq_f) or np.any(bk_f))
    with_v_bias = bool(np.any(bv_f))
    with_mask = bool(np.any(np.asarray(mask) == 0))

    key = (with_qk_bias, with_v_bias, with_mask)
    if key not in _GRAPH_CACHE:
        _GRAPH_CACHE[key] = _build_graph(*key)
    nc = _GRAPH_CACHE[key]

    if with_mask:
        maskT = np.ascontiguousarray(
            np.broadcast_to(mask, (1, 1, S, S))[0, 0].T.astype(ml_dtypes.bfloat16))

    in_maps = []
    for core in range(8):
        b, hg = core // 4, core % 4
        cs = slice(hg * D, (hg + 1) * D)
        m = {
            "xt": np.ascontiguousarray(x[b].T.astype(ml_dtypes.bfloat16)),
            "wq": np.ascontiguousarray(Mq[:, cs].astype(ml_dtypes.bfloat16)),
            "wk": np.ascontiguousarray(Mk[:, cs].astype(ml_dtypes.bfloat16)),
            "wv": np.ascontiguousarray(Mv[:, cs].astype(ml_dtypes.bfloat16)),
            "wo": np.ascontiguousarray(Mo[cs, :].astype(ml_dtypes.bfloat16)),
        }
        if with_qk_bias:
            m["bq"] = np.ascontiguousarray(bq_f[cs])
            m["bk"] = np.ascontiguousarray(bk_f[cs])
        if with_v_bias:
            m["bv"] = np.ascontiguousarray(bv_f[cs].astype(ml_dtypes.bfloat16))
        if with_mask:
            m["maskT"] = maskT
        in_maps.append(m)

    res = run_bass_kernel_spmd(nc, in_maps, core_ids=list(range(8))).results

    out = np.zeros((B, S, E), dtype=np.float32)
    for core in range(8):
        b = core // 4
        out[b] += res[core]["out"].T
    out += bo_f
    return out
